# revision 1
# baseline (speedup 1.0000x reference)
"""Bilinear causal attention (nn_Attention_34772055228779) on 8 trn2 cores.

reference:
  scores[i,k] = x[i] @ W_bi[k] @ x[i]          [512, 512]
  attn = softmax(scores + causal_mask, axis=1)
  out  = (attn @ x) @ W_out.T                  [512, 512]

Device strategy (tensor-parallel over score columns, per sharding hint):
  core m holds W_bi[64m:64(m+1)]  (64 MiB fp32)
  stage A: for each local k: Y_k = X @ W_k  (fp32r matmuls, lhsT = X^T resident)
           scores[:, k] = rowsum(Y_k * X)   (fused DVE scalar_tensor_tensor)
  AllToAll over the [8 x 64-row, 64-col] score shard blocks: core m ends up
           with rows [64m, 64m+64) of the FULL score matrix.
  tail:    masked softmax rows (ACT exp with fused accum), A^T via PE
           transpose, O^T = X^T A^T, Y = O @ W_out^T, DMA 64 output rows.
  host:    concatenates the 8 row blocks.
"""
import numpy as np

N_CTX = 512
D = 512
NCORES = 8
KSH = N_CTX // NCORES      # 64 score columns per core
RSH = N_CTX // NCORES      # 64 output rows per core
NEG_INF = -1e30
STAGE_A = "causal"   # "causal" skips fully-masked row-tiles (k-interleaved)

_nc_cache = None


def _build(timing_loop=0, use_collective=True, num_devices=NCORES,
           stage_a="base", wbufs=4, wgroup=1):
    """Build the Bass module.

    timing_loop=R>0 wraps the whole per-core body in a hardware For_i loop
    (R iterations) for slope timing; collectives can't sit in control flow,
    so timing variants pass use_collective=False (the gather DMA then reads
    the pre-collective buffer -- wrong data, identical shapes/costs).
    """
    import concourse.mybir as mybir
    import concourse.tile as tile
    from concourse import bacc

    f32 = mybir.dt.float32
    f32r = mybir.dt.float32r
    Alu = mybir.AluOpType
    Act = mybir.ActivationFunctionType

    nc = bacc.Bacc(
        "TRN2", target_bir_lowering=False, debug=False,
        enable_asserts=False, num_devices=num_devices,
    )

    x_t = nc.dram_tensor("x", [N_CTX, D], f32, kind="ExternalInput").ap()
    # column-permuted X rows (k-interleaved layout) for the attn @ X matmul
    xp_t = nc.dram_tensor("xperm", [N_CTX, D], f32, kind="ExternalInput").ap()
    xt_t = nc.dram_tensor("xt", [D, N_CTX], f32, kind="ExternalInput").ap()
    wbi_t = nc.dram_tensor("wbi", [KSH, D, D], f32, kind="ExternalInput").ap()
    woutt_t = nc.dram_tensor("wout_t", [D, D], f32, kind="ExternalInput").ap()
    mask_t = nc.dram_tensor("mask", [RSH, N_CTX], f32, kind="ExternalInput").ap()
    ident_t = nc.dram_tensor("ident", [128, 128], f32, kind="ExternalInput").ap()
    out_t = nc.dram_tensor("out", [RSH, D], f32, kind="ExternalOutput").ap()

    with tile.TileContext(nc) as tc:
        with (
            tc.tile_pool(name="const", bufs=1) as cpool,
            tc.tile_pool(name="wstream", bufs=wbufs) as wpool,
            tc.tile_pool(name="scratch", bufs=3) as spool,
            tc.tile_pool(name="small", bufs=1) as mpool,
            tc.tile_pool(name="psA", bufs=6, space="PSUM") as ppA,
            tc.tile_pool(name="psB", bufs=2, space="PSUM") as ppB,
            tc.tile_pool(name="dram", bufs=1, space="DRAM") as dpool,
        ):
            # ---- resident loads (outside any timing loop) -----------------
            x_sb, xt_sb, woutt_sb = [], [], []
            for t in range(4):
                b = cpool.tile([128, N_CTX], f32r, tag=f"xt{t}", name=f"xt{t}")
                nc.sync.dma_start(b[:], xt_t[t * 128:(t + 1) * 128, :].bitcast(f32r))
                xt_sb.append(b)
            xp_sb = []
            for t in range(4):
                a = cpool.tile([128, N_CTX], f32, tag=f"x{t}", name=f"x{t}")
                nc.sync.dma_start(a[:], x_t[t * 128:(t + 1) * 128, :])
                x_sb.append(a)
                d = cpool.tile([128, N_CTX], f32, tag=f"xp{t}", name=f"xp{t}")
                xp_sb.append(d)
                c = cpool.tile([128, D], f32, tag=f"wo{t}", name=f"wo{t}")
                woutt_sb.append(c)
            mask_sb = cpool.tile([RSH, N_CTX], f32, tag="mask")
            ident_sb = cpool.tile([128, 128], f32, tag="ident")
            scores_sb = [
                cpool.tile([128, KSH], f32, tag=f"sc{t}", name=f"sc{t}")
                for t in range(4)
            ]
            if stage_a == "causal":
                # skipped (nt, kk) cells are never written; zero them so no
                # NaN bit-patterns survive into exp() past the additive mask
                for t in range(4):
                    nc.gpsimd.memset(scores_sb[t][:], 0.0)
            agin = dpool.tile([N_CTX, KSH], f32, tag="agin")
            agout = dpool.tile([N_CTX, KSH], f32, tag="agout")

            def load_wk(kk):
                wk = wpool.tile([128, 4, D], f32r, tag="wk", name="wk")
                nc.sync.dma_start(
                    wk[:],
                    wbi_t[kk].rearrange("(dt p) e -> p dt e", p=128).bitcast(f32r),
                )
                return wk

            def load_wk_group(kb):
                # one big DMA for `wgroup` consecutive local k's (better
                # HBM efficiency than 1 MiB transfers)
                wk = wpool.tile([128, wgroup * 4, D], f32r, tag="wk", name="wk")
                nc.sync.dma_start(
                    wk[:],
                    wbi_t[kb:kb + wgroup]
                    .rearrange("g (dt p) e -> p (g dt) e", p=128)
                    .bitcast(f32r),
                )
                return wk

            def emit_stt(yp, nt, kk):
                scr = spool.tile([128, D], f32, tag="stt_out", name="scr")
                nc.vector.scalar_tensor_tensor(
                    out=scr[:], in0=yp[:], scalar=1.0, in1=x_sb[nt][:],
                    op0=Alu.mult, op1=Alu.mult,
                    accum_out=scores_sb[nt][:, kk:kk + 1],
                )

            def stage_a_base():
                # causal: with k-interleaved sharding (global k = 8*kk + m),
                # row-tiles nt < kk//16 are fully masked for column kk on
                # EVERY core, so the skip bound is SPMD-uniform.
                for kb in range(0, KSH, wgroup):
                    wk = load_wk_group(kb) if wgroup > 1 else load_wk(kb)
                    for g in range(wgroup):
                        kk = kb + g
                        nt_lo = (kk // 16) if stage_a == "causal" else 0
                        for nt in range(nt_lo, 4):
                            yp = ppA.tile([128, D], f32, tag="yp", name="yp")
                            for dt in range(4):
                                nc.tensor.matmul(
                                    yp[:],
                                    lhsT=xt_sb[dt][:, nt * 128:(nt + 1) * 128],
                                    rhs=wk[:, g * 4 + dt, :],
                                    start=(dt == 0),
                                    stop=(dt == 3),
                                )
                            emit_stt(yp, nt, kk)

            def stage_a_kpair():
                # process k in pairs; consecutive matmuls share the same
                # stationary lhsT tile (halves PE weight reloads)
                for kk in range(0, KSH, 2):
                    wk0 = load_wk(kk)
                    wk1 = load_wk(kk + 1)
                    for nt in range(4):
                        yp0 = ppA.tile([128, D], f32, tag="yp", name="yp")
                        yp1 = ppA.tile([128, D], f32, tag="yp", name="yp")
                        for dt in range(4):
                            lhsT = xt_sb[dt][:, nt * 128:(nt + 1) * 128]
                            nc.tensor.matmul(
                                yp0[:], lhsT=lhsT, rhs=wk0[:, dt, :],
                                start=(dt == 0), stop=(dt == 3),
                                skip_group_check=True)
                            nc.tensor.matmul(
                                yp1[:], lhsT=lhsT, rhs=wk1[:, dt, :],
                                start=(dt == 0), stop=(dt == 3),
                                skip_group_check=True)
                        emit_stt(yp0, nt, kk)
                        emit_stt(yp1, nt, kk + 1)

            def body():
                # ---- stage A: local score columns -------------------------
                if stage_a == "kpair":
                    stage_a_kpair()
                else:
                    stage_a_base()

                # tail-only constants: emitted after stage A so their DMAs
                # don't delay the first W_k prefetches
                nc.sync.dma_start(mask_sb[:], mask_t[:])
                nc.sync.dma_start(ident_sb[:], ident_t[:])
                for t in range(4):
                    nc.sync.dma_start(
                        woutt_sb[t][:], woutt_t[t * 128:(t + 1) * 128, :])
                    nc.sync.dma_start(
                        xp_sb[t][:], xp_t[t * 128:(t + 1) * 128, :])

                # ---- AllToAll: shard columns -> shard rows ----------------
                for nt in range(4):
                    nc.sync.dma_start(
                        agin[nt * 128:(nt + 1) * 128, :], scores_sb[nt][:])
                if use_collective:
                    nc.gpsimd.collective_compute(
                        "AllToAll",
                        mybir.AluOpType.bypass,
                        replica_groups=[list(range(NCORES))],
                        ins=[agin[:].opt()],
                        outs=[agout[:].opt()],
                    )
                    coll_out = agout
                else:
                    coll_out = agin
                # rows of the full score matrix for this core: [64, 512]
                sfull = mpool.tile([RSH, N_CTX], f32, tag="sfull", name="sfull")
                nc.sync.dma_start(
                    sfull[:].rearrange("i (r k) -> i r k", r=NCORES),
                    coll_out[:].rearrange("(r i) k -> i r k", r=NCORES),
                )

                # ---- masked softmax over the 64 rows ----------------------
                sm = mpool.tile([RSH, N_CTX], f32, tag="sm", name="sm")
                nc.vector.tensor_tensor(
                    out=sm[:], in0=sfull[:], in1=mask_sb[:], op=Alu.add)
                negm = mpool.tile([RSH, 1], f32, tag="negm", name="negm")
                nc.vector.reduce_max(negm[:], sm[:], axis=mybir.AxisListType.X,
                                     negate=True)
                esb = mpool.tile([RSH, N_CTX], f32, tag="esb", name="esb")
                den = mpool.tile([RSH, 1], f32, tag="den", name="den")
                nc.scalar.activation(
                    esb[:], sm[:], Act.Exp, bias=negm[:], scale=1.0,
                    accum_out=den[:])
                rden = mpool.tile([RSH, 1], f32, tag="rden", name="rden")
                nc.vector.reciprocal(rden[:], den[:])
                a_sb = mpool.tile([RSH, N_CTX], f32, tag="a_sb", name="a_sb")
                nc.vector.tensor_scalar_mul(a_sb[:], esb[:], rden[:])

                # ---- A^T via PE transpose: [64, 512] -> 4x [128, 64] ------
                at_sb = []
                for kt in range(4):
                    tp = ppB.tile([128, 512], f32, tag="tail", name="tp")
                    nc.tensor.transpose(
                        tp[:, 0:RSH],
                        a_sb[:, kt * 128:(kt + 1) * 128],
                        ident_sb[0:RSH, 0:RSH],
                    )
                    at = mpool.tile([128, RSH], f32, tag=f"at{kt}", name=f"at{kt}")
                    nc.scalar.copy(at[:], tp[:, 0:RSH])
                    at_sb.append(at)

                # ---- O^T = X^T @ A^T : [512(e), 64(i)] --------------------
                ot_sb = []
                for et in range(4):
                    op = ppB.tile([128, 512], f32, tag="tail", name="op")
                    for kt in range(4):
                        nc.tensor.matmul(
                            op[:, 0:RSH],
                            lhsT=xp_sb[kt][:, et * 128:(et + 1) * 128],
                            rhs=at_sb[kt][:],
                            start=(kt == 0),
                            stop=(kt == 3),
                        )
                    ot = mpool.tile([128, RSH], f32, tag=f"ot{et}", name=f"ot{et}")
                    nc.scalar.copy(ot[:], op[:, 0:RSH])
                    ot_sb.append(ot)

                # ---- Y = O @ W_out^T : [64(i), 512(f)] --------------------
                ypz = ppB.tile([128, 512], f32, tag="tail", name="ypz")
                for et in range(4):
                    nc.tensor.matmul(
                        ypz[0:RSH, :],
                        lhsT=ot_sb[et][:],
                        rhs=woutt_sb[et][:],
                        start=(et == 0),
                        stop=(et == 3),
                    )
                y_sb = mpool.tile([RSH, D], f32, tag="y_sb", name="y_sb")
                nc.scalar.copy(y_sb[:], ypz[0:RSH, :])
                nc.sync.dma_start(out_t[:], y_sb[:])

            if timing_loop:
                with tc.For_i(0, timing_loop, 1):
                    body()
            else:
                body()

    nc.compile()
    return nc


def _make_in_maps(x, W_bi, W_out, stage_a="causal"):
    x = np.ascontiguousarray(np.asarray(x, dtype=np.float32))
    W_bi = np.asarray(W_bi, dtype=np.float32)
    W_out = np.asarray(W_out, dtype=np.float32)
    xt = np.ascontiguousarray(x.T)
    woutt = np.ascontiguousarray(W_out.T)
    ident = np.eye(128, dtype=np.float32)
    if stage_a == "causal":
        # interleaved k-sharding: core m owns global columns k = 8*kk + m.
        # After the AllToAll gather, score column position p = r*64 + kk
        # holds global k = 8*kk + r, so X rows and the causal mask are
        # permuted to match.
        perm = np.array([8 * (p % KSH) + p // KSH for p in range(N_CTX)])
        xperm = np.ascontiguousarray(x[perm])
        kcol = perm[None, :]                       # global k at position p
        shards = [np.ascontiguousarray(W_bi[m::NCORES]) for m in range(NCORES)]
    else:
        perm = np.arange(N_CTX)
        xperm = x
        kcol = perm[None, :]
        shards = [np.ascontiguousarray(W_bi[m * KSH:(m + 1) * KSH])
                  for m in range(NCORES)]
    in_maps = []
    for m in range(NCORES):
        rows = np.arange(m * RSH, (m + 1) * RSH)[:, None]
        mask = np.where(kcol <= rows, 0.0, NEG_INF).astype(np.float32)
        in_maps.append({
            "x": x,
            "xperm": xperm,
            "xt": xt,
            "wbi": shards[m],
            "wout_t": woutt,
            "mask": np.ascontiguousarray(mask),
            "ident": ident,
        })
    return in_maps


def kernel(x, W_bi, W_out):
    global _nc_cache
    import time as _time
    from concourse.bass_utils import run_bass_kernel_spmd

    if _nc_cache is None:
        _nc_cache = _build(stage_a=STAGE_A)
    nc = _nc_cache
    in_maps = _make_in_maps(x, W_bi, W_out, stage_a=STAGE_A)
    last_exc = None
    for attempt in range(3):
        try:
            res = run_bass_kernel_spmd(nc, in_maps, core_ids=list(range(NCORES)),
                                       trace=False)
            break
        except Exception as e:  # transient NRT/axon wedges recover on retry
            last_exc = e
            _time.sleep(5.0 * (attempt + 1))
    else:
        raise last_exc
    out = np.concatenate([res.results[m]["out"] for m in range(NCORES)], axis=0)
    return np.ascontiguousarray(out, dtype=np.float32)



# revision 37
# speedup vs baseline: 1.2651x; 1.2651x over previous
"""Bilinear causal attention (nn_Attention_34772055228779) on 8 trn2 cores.

reference:
  scores[i,k] = x[i] @ W_bi[k] @ x[i]          [512, 512]
  attn = softmax(scores + causal_mask, axis=1)
  out  = (attn @ x) @ W_out.T                  [512, 512]

Device strategy (tensor-parallel over score columns, per sharding hint):
  core m holds the k-interleaved shard W_bi[m::8] (64 local columns).

  Only the symmetric part of W_bi[k] contributes to x^T W x, so the host
  packs U'_k = triu(W_k + W_k^T, 1) + diag(W_k)  (exact identity:
  x^T U' x = x^T W x).  U' is upper-triangular, so the d-row-block dt only
  has nonzeros in columns e >= 128*dt: the four matmul rhs spans are
  512/384/256/128 instead of 4x512 (37.5%% less PE work), and the packed
  fp16 stream is 320 KiB/k = 20 MiB/core instead of 64 MiB fp32.

  stage A: for each local k: Y_k = X16 @ U'16_k (fp16 matmuls, fp32 PSUM,
           lhsT = X^T fp16 resident), scores[:, k] = rowsum(Y_k * X32)
           via scalar_tensor_tensor, alternating DVE / Pool(gpsimd) so
           neither vector engine is the bottleneck.
  AllToAll over the [8 x 64-row, 64-col] score shard blocks: core m ends up
           with rows [64m, 64m+64) of the FULL score matrix.
  tail:    masked softmax rows (ACT exp with fused accum), A^T via fp16 PE
           transpose, O^T = X^T A^T, Y = O @ W_out^T (fp16), DMA 64 rows.
  host:    concatenates the 8 row blocks.
"""
import numpy as np

N_CTX = 512
D = 512
NCORES = 8
KSH = N_CTX // NCORES      # 64 score columns per core
RSH = N_CTX // NCORES      # 64 output rows per core
NEG_INF = -1e30
STAGE_A = "causal"   # "causal" skips fully-masked row-tiles (k-interleaved)

# upper-triangular pack: per dt row-block, columns [128*dt, 512)
SPANS = [512, 384, 256, 128]
OFFS = [0, 512, 896, 1152]          # column offset of block dt in the pack
PACKW = 1280                         # total packed width per partition

_nc_cache = None


def _build(timing_loop=0, use_collective=True, num_devices=NCORES,
           stage_a="causal", wbufs=4, stt_split=True, softmax_fused=False,
           gather_3d=True, debug_scores=False):
    # NOTE: softmax_fused=True (tensor_tensor_reduce min) compiles but
    # crashes the exec unit on real TRN2 hardware -- keep it off.
    """Build the Bass module.

    timing_loop=R>0 wraps the whole per-core body in a hardware For_i loop
    (R iterations) for slope timing; collectives can't sit in control flow,
    so timing variants pass use_collective=False (the gather DMA then reads
    the pre-collective buffer -- wrong data, identical shapes/costs).
    """
    import concourse.mybir as mybir
    import concourse.tile as tile
    from concourse import bacc

    f32 = mybir.dt.float32
    f16 = mybir.dt.float16
    Alu = mybir.AluOpType
    Act = mybir.ActivationFunctionType

    nc = bacc.Bacc(
        "TRN2", target_bir_lowering=False, debug=False,
        enable_asserts=False, num_devices=num_devices,
    )

    x_t = nc.dram_tensor("x", [N_CTX, D], f32, kind="ExternalInput").ap()
    x16_t = nc.dram_tensor("x16", [N_CTX, D], f16, kind="ExternalInput").ap()
    # column-permuted X rows (k-interleaved layout) for the attn @ X matmul
    xp_t = nc.dram_tensor("xperm", [N_CTX, D], f16, kind="ExternalInput").ap()
    # X^T packed [p, dt, n]: one DMA loads all four lhsT d-blocks
    xtp_t = nc.dram_tensor("xtp", [128, 4, N_CTX], f16,
                           kind="ExternalInput").ap()
    wbi_t = nc.dram_tensor("wbi", [KSH, 128, PACKW], f16,
                           kind="ExternalInput").ap()
    woutt_t = nc.dram_tensor("wout_t", [D, D], f16, kind="ExternalInput").ap()
    # negated additive mask: 0 where allowed, +1e30 where causally masked
    # (the softmax uses nsm = negmask - scores and a min-reduction)
    mask_t = nc.dram_tensor("mask", [RSH, N_CTX], f32, kind="ExternalInput").ap()
    ident_t = nc.dram_tensor("ident", [128, 128], f32, kind="ExternalInput").ap()
    out_t = nc.dram_tensor("out", [RSH, D], f32, kind="ExternalOutput").ap()
    dbg_t = (nc.dram_tensor("dbg", [128, 4 * KSH], f32,
                            kind="ExternalOutput").ap()
             if debug_scores else None)

    with tile.TileContext(nc) as tc:
        with (
            tc.tile_pool(name="const", bufs=1) as cpool,
            tc.tile_pool(name="wstream", bufs=wbufs) as wpool,
            tc.tile_pool(name="scratch", bufs=3) as spool,
            tc.tile_pool(name="scratch2", bufs=3) as spool2,
            tc.tile_pool(name="small", bufs=1) as mpool,
            tc.tile_pool(name="psA", bufs=6, space="PSUM") as ppA,
            tc.tile_pool(name="psB", bufs=2, space="PSUM") as ppB,
            tc.tile_pool(name="dram", bufs=1, space="DRAM") as dpool,
        ):
            # ---- resident loads (outside any timing loop) -----------------
            # xt first (single packed DMA): the first matmul only needs
            # xt + wk0, so the x/x16 loads (needed ~2.6us later by the
            # first stt) are issued after the first wk DMAs to cut the
            # startup serial chain.
            xtp_sb = cpool.tile([128, 4, N_CTX], f16, tag="xtp", name="xtp")
            nc.sync.dma_start(xtp_sb[:], xtp_t[:])
            x_sb, x16_sb, woutt_sb, xp_sb = [], [], [], []
            for t in range(4):
                a = cpool.tile([128, N_CTX], f32, tag=f"x{t}", name=f"x{t}")
                x_sb.append(a)
                a16 = cpool.tile([128, N_CTX], f16, tag=f"x16{t}",
                                 name=f"x16{t}")
                x16_sb.append(a16)
                d = cpool.tile([128, N_CTX], f16, tag=f"xp{t}", name=f"xp{t}")
                xp_sb.append(d)
                c = cpool.tile([128, D], f16, tag=f"wo{t}", name=f"wo{t}")
                woutt_sb.append(c)

            def load_x_resident():
                for t in range(4):
                    nc.sync.dma_start(
                        x_sb[t][:], x_t[t * 128:(t + 1) * 128, :])
                    nc.sync.dma_start(
                        x16_sb[t][:], x16_t[t * 128:(t + 1) * 128, :])
            mask_sb = cpool.tile([RSH, N_CTX], f32, tag="mask")
            ident_sb = cpool.tile([128, 128], f32, tag="ident")
            # single score accumulator tile, column nt*KSH + kk
            scores_sb = cpool.tile([128, 4 * KSH], f32, tag="sc", name="sc")
            # skipped (nt, kk) cells are never written; zero them so no
            # NaN bit-patterns survive into exp() past the additive mask
            nc.gpsimd.memset(scores_sb[:], 0.0)
            agin = dpool.tile([N_CTX, KSH], f32, tag="agin")
            agout = dpool.tile([N_CTX, KSH], f32, tag="agout")
            agin_v = agin[:].rearrange("(t p) k -> p t k", p=128)
            scores_v = scores_sb[:].rearrange("p (t k) -> p t k", t=4)

            def load_wk(kk):
                wk = wpool.tile([128, PACKW], f16, tag="wk", name="wk")
                nc.sync.dma_start(wk[:], wbi_t[kk])
                return wk

            # stt engine split: only DVE can reduce straight from PSUM
            # (Pool has no PSUM access and TensorScalarPtr is not a legal
            # Pool opcode).  A share of tiles is therefore routed
            #   ACT:  yp (PSUM f32) -> y16 (SBUF f16)
            #   Pool: prod16 = y16 * x16          (TensorTensor, SBUF)
            #   ACT:  Copy(prod16) with accum_out -> scores column
            # Costs: DVE stt ~658 ns; ACT ~610+610 ns and Pool ~840 ns per
            # routed tile.  A 13:7 split puts DVE ~68us, ACT ~68us and
            # Pool ~47us, all under the ~90us PE stage-A floor.
            stt_state = {"i": 0}

            def emit_stt(yp, nt, kk):
                if stt_split:
                    use_dve = (stt_state["i"] % 20) < 13
                    stt_state["i"] += 1
                else:
                    use_dve = True
                col = nt * KSH + kk
                if use_dve:
                    scr = spool.tile([128, D], f32, tag="stt_out", name="scr")
                    nc.vector.scalar_tensor_tensor(
                        out=scr[:], in0=yp[:], scalar=1.0, in1=x_sb[nt][:],
                        op0=Alu.mult, op1=Alu.mult,
                        accum_out=scores_sb[:, col:col + 1],
                    )
                else:
                    y16 = spool2.tile([128, D], f16, tag="y16", name="y16")
                    nc.scalar.copy(y16[:], yp[:])
                    prod = spool2.tile([128, D], f16, tag="prod", name="prod")
                    nc.gpsimd.tensor_tensor(
                        out=prod[:], in0=y16[:], in1=x16_sb[nt][:],
                        op=Alu.mult)
                    scr = spool2.tile([128, D], f16, tag="scr16", name="scr16")
                    nc.scalar.activation(
                        scr[:], prod[:], Act.Copy, bias=0.0, scale=1.0,
                        accum_out=scores_sb[:, col:col + 1])

            def stage_a_tri():
                # causal: with k-interleaved sharding (global k = 8*kk + m),
                # row-tiles nt < kk//16 are fully masked for column kk on
                # EVERY core, so the skip bound is SPMD-uniform.
                #
                # Column order pairs kk with 63-kk: every pair is exactly 5
                # kept row-tiles of PE work against 2 wk DMAs, so the DMA
                # stream never outpaces nor starves the PE (a plain
                # ascending order leaves PE idle behind DMA for the late,
                # 1-tile columns).
                order = []
                for j in range(KSH // 2):
                    order += [j, KSH - 1 - j]
                for idx, kk in enumerate(order):
                    wk = load_wk(kk)
                    if idx == 0:
                        # must precede the first stt in program order: the
                        # dependency tracker only orders reads after writes
                        # that were already emitted
                        load_x_resident()
                    nt_lo = (kk // 16) if stage_a == "causal" else 0
                    for nt in range(nt_lo, 4):
                        yp = ppA.tile([128, D], f32, tag="yp", name="yp")
                        for dt in range(4):
                            span = SPANS[dt]
                            nc.tensor.matmul(
                                yp[:, D - span:D],
                                lhsT=xtp_sb[:, dt, nt * 128:(nt + 1) * 128],
                                rhs=wk[:, OFFS[dt]:OFFS[dt] + span],
                                start=(dt == 0),
                                stop=(dt == 3),
                                skip_group_check=True,
                            )
                        emit_stt(yp, nt, kk)
                    if idx == 31 and gather_3d:
                        # columns {0..15, 48..63} are final: start their
                        # DRAM gather under the remaining compute
                        nc.sync.dma_start(
                            agin_v[:, :, 0:16], scores_v[:, :, 0:16])
                        nc.sync.dma_start(
                            agin_v[:, :, 48:64], scores_v[:, :, 48:64])

            def body():
                # ---- stage A: local score columns -------------------------
                stage_a_tri()

                # tail-only constants: emitted after stage A so their DMAs
                # don't delay the first W_k prefetches
                nc.sync.dma_start(mask_sb[:], mask_t[:])
                nc.sync.dma_start(ident_sb[:], ident_t[:])
                for t in range(4):
                    nc.sync.dma_start(
                        woutt_sb[t][:], woutt_t[t * 128:(t + 1) * 128, :])
                    nc.sync.dma_start(
                        xp_sb[t][:], xp_t[t * 128:(t + 1) * 128, :])

                # ---- AllToAll: shard columns -> shard rows ----------------
                # (columns {0..15, 48..63} were already gathered mid-stage-A)
                if gather_3d:
                    nc.sync.dma_start(
                        agin_v[:, :, 16:48], scores_v[:, :, 16:48])
                else:
                    for nt in range(4):
                        nc.sync.dma_start(
                            agin[nt * 128:(nt + 1) * 128, :],
                            scores_sb[:, nt * KSH:(nt + 1) * KSH])
                if use_collective:
                    nc.gpsimd.collective_compute(
                        "AllToAll",
                        mybir.AluOpType.bypass,
                        replica_groups=[list(range(NCORES))],
                        ins=[agin[:].opt()],
                        outs=[agout[:].opt()],
                    )
                    coll_out = agout
                else:
                    coll_out = agin
                # rows of the full score matrix for this core: [64, 512]
                sfull = mpool.tile([RSH, N_CTX], f32, tag="sfull", name="sfull")
                nc.sync.dma_start(
                    sfull[:].rearrange("i (r k) -> i r k", r=NCORES),
                    coll_out[:].rearrange("(r i) k -> i r k", r=NCORES),
                )

                # ---- masked softmax over the 64 rows ----------------------
                # fused mask+max: nsm = negmask - scores (so masked cells are
                # ~+1e30 and min(nsm) = -max of the allowed scores), then
                # exp(-nsm + bias) on ACT.  The 1/denominator is folded into
                # the final output copy as a per-partition ACT scale, keeping
                # the reciprocal off the critical path.
                nsm = mpool.tile([RSH, N_CTX], f32, tag="sm", name="sm")
                negm = mpool.tile([RSH, 1], f32, tag="negm", name="negm")
                esb = mpool.tile([RSH, N_CTX], f32, tag="esb", name="esb")
                den = mpool.tile([RSH, 1], f32, tag="den", name="den")
                if softmax_fused:
                    # nsm = negmask - s (masked cells ~ +1e30), negm =
                    # min(nsm) = -max over allowed, exp(-nsm + negm)
                    nc.vector.tensor_tensor_reduce(
                        out=nsm[:], in0=mask_sb[:], in1=sfull[:], scale=1.0,
                        scalar=float(-NEG_INF), op0=Alu.subtract, op1=Alu.min,
                        accum_out=negm[:])
                    nc.scalar.activation(
                        esb[:], nsm[:], Act.Exp, bias=negm[:], scale=-1.0,
                        accum_out=den[:])
                else:
                    # sm = s - negmask (masked cells ~ -1e30)
                    nc.vector.tensor_tensor(
                        out=nsm[:], in0=sfull[:], in1=mask_sb[:],
                        op=Alu.subtract)
                    nc.vector.reduce_max(
                        negm[:], nsm[:], axis=mybir.AxisListType.X,
                        negate=True)
                    nc.scalar.activation(
                        esb[:], nsm[:], Act.Exp, bias=negm[:], scale=1.0,
                        accum_out=den[:])
                rden = mpool.tile([RSH, 1], f32, tag="rden", name="rden")
                nc.vector.reciprocal(rden[:], den[:])

                # ---- A^T via PE transpose: [64, 512] -> 4x [128, 64] ------
                # (unnormalized exp weights; f32 transpose, and the
                # PSUM->SBUF copy casts to fp16 for the fp16 matmuls)
                at_sb = []
                for kt in range(4):
                    tp = ppB.tile([128, 512], f32, tag="tail", name="tp")
                    nc.tensor.transpose(
                        tp[:, 0:RSH],
                        esb[:, kt * 128:(kt + 1) * 128],
                        ident_sb[0:RSH, 0:RSH],
                    )
                    at = mpool.tile([128, RSH], f16, tag=f"at{kt}",
                                    name=f"at{kt}")
                    nc.scalar.copy(at[:], tp[:, 0:RSH])
                    at_sb.append(at)

                # ---- O^T = X^T @ A^T : [512(e), 64(i)] --------------------
                ot_sb = []
                for et in range(4):
                    op = ppB.tile([128, 512], f32, tag="tail", name="op")
                    for kt in range(4):
                        nc.tensor.matmul(
                            op[:, 0:RSH],
                            lhsT=xp_sb[kt][:, et * 128:(et + 1) * 128],
                            rhs=at_sb[kt][:],
                            start=(kt == 0),
                            stop=(kt == 3),
                        )
                    ot = mpool.tile([128, RSH], f16, tag=f"ot{et}",
                                    name=f"ot{et}")
                    nc.scalar.copy(ot[:], op[:, 0:RSH])
                    ot_sb.append(ot)

                # ---- Y = O @ W_out^T : [64(i), 512(f)] --------------------
                ypz = ppB.tile([128, 512], f32, tag="tail", name="ypz")
                for et in range(4):
                    nc.tensor.matmul(
                        ypz[0:RSH, :],
                        lhsT=ot_sb[et][:],
                        rhs=woutt_sb[et][:],
                        start=(et == 0),
                        stop=(et == 3),
                    )
                # final copy normalizes the softmax: per-partition 1/den
                y_sb = mpool.tile([RSH, D], f32, tag="y_sb", name="y_sb")
                nc.scalar.mul(y_sb[:], ypz[0:RSH, :], rden[:])
                nc.sync.dma_start(out_t[:], y_sb[:])
                if debug_scores:
                    nc.sync.dma_start(dbg_t[:], scores_sb[:])

            if timing_loop:
                with tc.For_i(0, timing_loop, 1):
                    body()
            else:
                body()

    nc.compile()
    return nc


def _pack_upper(Wm):
    """[KSH, 512, 512] fp32 -> [KSH, 128, PACKW] fp16 upper-tri pack.

    U' = triu(W + W^T, 1) + diag(W); block dt holds U'[128dt+p, 128dt:512].
    """
    U = np.triu(Wm + Wm.transpose(0, 2, 1), 1)
    idx = np.arange(D)
    U[:, idx, idx] = Wm[:, idx, idx]
    pack = np.empty((KSH, 128, PACKW), np.float16)
    for dt in range(4):
        lo = 128 * dt
        pack[:, :, OFFS[dt]:OFFS[dt] + SPANS[dt]] = U[:, lo:lo + 128, lo:D]
    return pack


def _make_in_maps(x, W_bi, W_out, stage_a="causal"):
    x = np.ascontiguousarray(np.asarray(x, dtype=np.float32))
    W_bi = np.asarray(W_bi, dtype=np.float32)
    W_out = np.asarray(W_out, dtype=np.float32)
    x16 = x.astype(np.float16)
    # xtp[p, dt, n] = x[n, 128*dt + p]
    xtp16 = np.ascontiguousarray(
        x.T.reshape(4, 128, N_CTX).transpose(1, 0, 2)).astype(np.float16)
    woutt16 = np.ascontiguousarray(W_out.T).astype(np.float16)
    ident = np.eye(128, dtype=np.float32)
    # interleaved k-sharding: core m owns global columns k = 8*kk + m.
    # After the AllToAll gather, score column position p = r*64 + kk
    # holds global k = 8*kk + r, so X rows and the causal mask are
    # permuted to match.
    perm = np.array([8 * (p % KSH) + p // KSH for p in range(N_CTX)])
    xperm16 = np.ascontiguousarray(x[perm]).astype(np.float16)
    kcol = perm[None, :]                       # global k at position p
    shards = [_pack_upper(np.ascontiguousarray(W_bi[m::NCORES]))
              for m in range(NCORES)]
    in_maps = []
    for m in range(NCORES):
        rows = np.arange(m * RSH, (m + 1) * RSH)[:, None]
        # negated mask: 0 where allowed, +1e30 where masked
        mask = np.where(kcol <= rows, 0.0, -NEG_INF).astype(np.float32)
        in_maps.append({
            "x": x,
            "x16": x16,
            "xperm": xperm16,
            "xtp": xtp16,
            "wbi": shards[m],
            "wout_t": woutt16,
            "mask": np.ascontiguousarray(mask),
            "ident": ident,
        })
    return in_maps


def kernel(x, W_bi, W_out):
    global _nc_cache
    import time as _time
    from concourse.bass_utils import run_bass_kernel_spmd

    if _nc_cache is None:
        _nc_cache = _build(stage_a=STAGE_A)
    nc = _nc_cache
    in_maps = _make_in_maps(x, W_bi, W_out, stage_a=STAGE_A)
    last_exc = None
    for attempt in range(3):
        try:
            res = run_bass_kernel_spmd(nc, in_maps, core_ids=list(range(NCORES)),
                                       trace=False)
            break
        except Exception as e:  # transient NRT/axon wedges recover on retry
            last_exc = e
            _time.sleep(5.0 * (attempt + 1))
    else:
        raise last_exc
    out = np.concatenate([res.results[m]["out"] for m in range(NCORES)], axis=0)
    return np.ascontiguousarray(out, dtype=np.float32)


# revision 47
# speedup vs baseline: 1.7829x; 1.4093x over previous
"""Bilinear causal attention (nn_Attention_34772055228779) on 8 trn2 cores.

reference:
  scores[i,k] = x[i] @ W_bi[k] @ x[i]          [512, 512]
  attn = softmax(scores + causal_mask, axis=1)
  out  = (attn @ x) @ W_out.T                  [512, 512]

Device strategy (tensor-parallel over score columns, per sharding hint):
  core m holds the k-interleaved shard W_bi[m::8] (64 local columns).

  Only the symmetric part of W_bi[k] contributes to x^T W x, so the host
  packs U'_k = triu(W_k + W_k^T, 1) + diag(W_k)  (exact identity:
  x^T U' x = x^T W x).  U' is upper-triangular, so the d-row-block dt only
  has nonzeros in columns e >= 128*dt: the four matmul rhs spans are
  512/384/256/128 instead of 4x512 (37.5%% less PE work), and the packed
  fp16 stream is 320 KiB/k = 20 MiB/core instead of 64 MiB fp32.

  stage A: for each local k: Y_k = X16 @ U'16_k (fp16 matmuls, fp32 PSUM,
           lhsT = X^T fp16 resident), scores[:, k] = rowsum(Y_k * X32)
           via scalar_tensor_tensor, alternating DVE / Pool(gpsimd) so
           neither vector engine is the bottleneck.
  AllToAll over the [8 x 64-row, 64-col] score shard blocks: core m ends up
           with rows [64m, 64m+64) of the FULL score matrix.
  tail:    masked softmax rows (ACT exp with fused accum), A^T via fp16 PE
           transpose, O^T = X^T A^T, Y = O @ W_out^T (fp16), DMA 64 rows.
  host:    concatenates the 8 row blocks.
"""
import numpy as np

N_CTX = 512
D = 512
NCORES = 8
KSH = N_CTX // NCORES      # 64 score columns per core
RSH = N_CTX // NCORES      # 64 output rows per core
NEG_INF = -1e30
STAGE_A = "causal"   # "causal" skips fully-masked row-tiles (k-interleaved)

# upper-triangular pack: per dt row-block, columns [128*dt, 512)
SPANS = [512, 384, 256, 128]
OFFS = [0, 512, 896, 1152]          # column offset of block dt in the pack
PACKW = 1280                         # total packed width per partition

_nc_cache = None


def _build(timing_loop=0, use_collective=True, num_devices=NCORES,
           stage_a="causal", wbufs=4, stt_split=True, softmax_fused=False,
           gather_3d=True, debug_scores=False):
    # NOTE: softmax_fused=True (tensor_tensor_reduce min) compiles but
    # crashes the exec unit on real TRN2 hardware -- keep it off.
    """Build the Bass module.

    timing_loop=R>0 wraps the whole per-core body in a hardware For_i loop
    (R iterations) for slope timing; collectives can't sit in control flow,
    so timing variants pass use_collective=False (the gather DMA then reads
    the pre-collective buffer -- wrong data, identical shapes/costs).
    """
    import concourse.mybir as mybir
    import concourse.tile as tile
    from concourse import bacc

    f32 = mybir.dt.float32
    f16 = mybir.dt.float16
    Alu = mybir.AluOpType
    Act = mybir.ActivationFunctionType

    nc = bacc.Bacc(
        "TRN2", target_bir_lowering=False, debug=False,
        enable_asserts=False, num_devices=num_devices,
    )

    # x row-major packed [p, nt, d] in f32 and f16: one DMA each
    x_t = nc.dram_tensor("x", [128, 4, D], f32, kind="ExternalInput").ap()
    x16_t = nc.dram_tensor("x16", [128, 4, D], f16, kind="ExternalInput").ap()
    # tail constants packed [p, 8, e] f16: [:,0:4] = column-permuted X rows
    # (k-interleaved layout) for attn @ X, [:,4:8] = W_out^T blocks
    xpwo_t = nc.dram_tensor("xpwo", [128, 8, D], f16,
                            kind="ExternalInput").ap()
    # X^T packed [p, dt, n]: one DMA loads all four lhsT d-blocks
    xtp_t = nc.dram_tensor("xtp", [128, 4, N_CTX], f16,
                           kind="ExternalInput").ap()
    # W pairs: [j] holds packed U' for columns kk=j and kk=63-j
    wbi_t = nc.dram_tensor("wbi", [KSH // 2, 128, 2 * PACKW], f16,
                           kind="ExternalInput").ap()
    # negated additive mask: 0 where allowed, +1e30 where causally masked
    mask_t = nc.dram_tensor("mask", [RSH, N_CTX], f32, kind="ExternalInput").ap()
    ident_t = nc.dram_tensor("ident", [128, 128], f32, kind="ExternalInput").ap()
    out_t = nc.dram_tensor("out", [RSH, D], f32, kind="ExternalOutput").ap()
    dbg_t = (nc.dram_tensor("dbg", [128, 4 * KSH], f32,
                            kind="ExternalOutput").ap()
             if debug_scores else None)

    with tile.TileContext(nc) as tc:
        with (
            tc.tile_pool(name="const", bufs=1) as cpool,
            tc.tile_pool(name="tailc", bufs=2) as tcpool,
            tc.tile_pool(name="wstream", bufs=wbufs) as wpool,
            tc.tile_pool(name="scratch", bufs=3) as spool,
            tc.tile_pool(name="scratch2", bufs=3) as spool2,
            tc.tile_pool(name="small", bufs=1) as mpool,
            tc.tile_pool(name="psA", bufs=6, space="PSUM") as ppA,
            tc.tile_pool(name="psB", bufs=2, space="PSUM") as ppB,
            tc.tile_pool(name="dram", bufs=1, space="DRAM") as dpool,
        ):
            # ---- resident loads (outside any timing loop) -----------------
            # xt first (single packed DMA): the first matmul only needs
            # xt + wk0, so the x/x16 loads (needed ~2.6us later by the
            # first stt) are issued after the first wk DMA to cut the
            # startup serial chain.
            xtp_sb = cpool.tile([128, 4, N_CTX], f16, tag="xtp", name="xtp")
            nc.sync.dma_start(xtp_sb[:], xtp_t[:])
            xpk_sb = cpool.tile([128, 4, N_CTX], f32, tag="xpk", name="xpk")
            x16k_sb = cpool.tile([128, 4, N_CTX], f16, tag="x16k",
                                 name="x16k")

            def load_x_resident():
                nc.sync.dma_start(xpk_sb[:], x_t[:])
                nc.sync.dma_start(x16k_sb[:], x16_t[:])
            # single score accumulator tile, column nt*KSH + kk
            scores_sb = cpool.tile([128, 4 * KSH], f32, tag="sc", name="sc")
            # skipped (nt, kk) cells are never written; zero them so no
            # NaN bit-patterns survive into exp() past the additive mask
            nc.gpsimd.memset(scores_sb[:], 0.0)
            agin = dpool.tile([N_CTX, KSH], f32, tag="agin")
            agout = dpool.tile([N_CTX, KSH], f32, tag="agout")
            agin_v = agin[:].rearrange("(t p) k -> p t k", p=128)
            scores_v = scores_sb[:].rearrange("p (t k) -> p t k", t=4)

            def load_wk_pair(j):
                # one DMA covers both columns of the pair (j, 63-j)
                wk = wpool.tile([128, 2 * PACKW], f16, tag="wk", name="wk")
                nc.sync.dma_start(wk[:], wbi_t[j])
                return wk

            # stt engine split: only DVE can reduce straight from PSUM
            # (Pool has no PSUM access and TensorScalarPtr is not a legal
            # Pool opcode).  A share of tiles is therefore routed
            #   ACT:  yp (PSUM f32) -> y16 (SBUF f16)
            #   Pool: prod16 = y16 * x16          (TensorTensor, SBUF)
            #   ACT:  Copy(prod16) with accum_out -> scores column
            # Costs: DVE stt ~658 ns; ACT ~2x660 ns and Pool ~840 ns per
            # routed tile.  6 of every 20 tiles (spread, not consecutive,
            # so DVE never sits idle for long) puts DVE ~76us, ACT ~64us
            # and Pool ~40us, all under the ~90us PE stage-A floor.
            POOL_SLOTS = {3, 6, 9, 13, 16, 19}
            stt_state = {"i": 0}

            def emit_stt(yp, nt, kk):
                if stt_split:
                    use_dve = (stt_state["i"] % 20) not in POOL_SLOTS
                    stt_state["i"] += 1
                else:
                    use_dve = True
                col = nt * KSH + kk
                if use_dve:
                    scr = spool.tile([128, D], f32, tag="stt_out", name="scr")
                    nc.vector.scalar_tensor_tensor(
                        out=scr[:], in0=yp[:], scalar=1.0,
                        in1=xpk_sb[:, nt, :],
                        op0=Alu.mult, op1=Alu.mult,
                        accum_out=scores_sb[:, col:col + 1],
                    )
                else:
                    y16 = spool2.tile([128, D], f16, tag="y16", name="y16")
                    nc.scalar.copy(y16[:], yp[:])
                    prod = spool2.tile([128, D], f16, tag="prod", name="prod")
                    nc.gpsimd.tensor_tensor(
                        out=prod[:], in0=y16[:], in1=x16k_sb[:, nt, :],
                        op=Alu.mult)
                    scr = spool2.tile([128, D], f16, tag="scr16", name="scr16")
                    nc.scalar.activation(
                        scr[:], prod[:], Act.Copy, bias=0.0, scale=1.0,
                        accum_out=scores_sb[:, col:col + 1])

            def stage_a_tri(load_tail_consts):
                # causal: with k-interleaved sharding (global k = 8*kk + m),
                # row-tiles nt < kk//16 are fully masked for column kk on
                # EVERY core, so the skip bound is SPMD-uniform.
                #
                # Column order pairs kk with 63-kk: every pair is exactly 5
                # kept row-tiles of PE work against 2 wk DMAs, so the DMA
                # stream never outpaces nor starves the PE (a plain
                # ascending order leaves PE idle behind DMA for the late,
                # 1-tile columns).
                for j in range(KSH // 2):
                    wk = load_wk_pair(j)
                    if j == 0:
                        # must precede the first stt in program order: the
                        # dependency tracker only orders reads after writes
                        # that were already emitted
                        load_x_resident()
                    if j == 3:
                        load_tail_consts()
                    for half, kk in enumerate((j, KSH - 1 - j)):
                        base = half * PACKW
                        nt_lo = (kk // 16) if stage_a == "causal" else 0
                        for nt in range(nt_lo, 4):
                            yp = ppA.tile([128, D], f32, tag="yp", name="yp")
                            for dt in range(4):
                                span = SPANS[dt]
                                nc.tensor.matmul(
                                    yp[:, D - span:D],
                                    lhsT=xtp_sb[:, dt,
                                                nt * 128:(nt + 1) * 128],
                                    rhs=wk[:, base + OFFS[dt]:
                                           base + OFFS[dt] + span],
                                    start=(dt == 0),
                                    stop=(dt == 3),
                                    skip_group_check=True,
                                )
                            emit_stt(yp, nt, kk)
                    if j == 15 and gather_3d:
                        # columns {0..15, 48..63} are final: start their
                        # DRAM gather under the remaining compute
                        nc.sync.dma_start(
                            agin_v[:, :, 0:16], scores_v[:, :, 0:16])
                        nc.sync.dma_start(
                            agin_v[:, :, 48:64], scores_v[:, :, 48:64])

            def body():
                # tail constants, double-buffered (bufs=2) so the timing
                # loop's next iteration can re-load them without a
                # write-after-read stall against this iteration's tail
                tail_c = {}

                def load_tail_consts():
                    tail_c["xpwo"] = tcpool.tile(
                        [128, 8, N_CTX], f16, tag="xpwo", name="xpwo")
                    tail_c["mask"] = tcpool.tile(
                        [RSH, N_CTX], f32, tag="mask", name="mask")
                    tail_c["ident"] = tcpool.tile(
                        [128, 128], f32, tag="ident", name="ident")
                    nc.sync.dma_start(tail_c["xpwo"][:], xpwo_t[:])
                    nc.sync.dma_start(tail_c["mask"][:], mask_t[:])
                    nc.sync.dma_start(tail_c["ident"][:], ident_t[:])

                # ---- stage A: local score columns -------------------------
                stage_a_tri(load_tail_consts)
                xpwo_sb = tail_c["xpwo"]
                mask_sb = tail_c["mask"]
                ident_sb = tail_c["ident"]

                # ---- AllToAll: shard columns -> shard rows ----------------
                # (columns {0..15, 48..63} were already gathered mid-stage-A)
                if gather_3d:
                    nc.sync.dma_start(
                        agin_v[:, :, 16:48], scores_v[:, :, 16:48])
                else:
                    for nt in range(4):
                        nc.sync.dma_start(
                            agin[nt * 128:(nt + 1) * 128, :],
                            scores_sb[:, nt * KSH:(nt + 1) * KSH])
                if use_collective:
                    nc.gpsimd.collective_compute(
                        "AllToAll",
                        mybir.AluOpType.bypass,
                        replica_groups=[list(range(NCORES))],
                        ins=[agin[:].opt()],
                        outs=[agout[:].opt()],
                    )
                    coll_out = agout
                else:
                    coll_out = agin
                # rows of the full score matrix for this core: [64, 512]
                sfull = mpool.tile([RSH, N_CTX], f32, tag="sfull", name="sfull")
                nc.sync.dma_start(
                    sfull[:].rearrange("i (r k) -> i r k", r=NCORES),
                    coll_out[:].rearrange("(r i) k -> i r k", r=NCORES),
                )

                # ---- masked softmax over the 64 rows ----------------------
                # fused mask+max: nsm = negmask - scores (so masked cells are
                # ~+1e30 and min(nsm) = -max of the allowed scores), then
                # exp(-nsm + bias) on ACT.  The 1/denominator is folded into
                # the final output copy as a per-partition ACT scale, keeping
                # the reciprocal off the critical path.
                nsm = mpool.tile([RSH, N_CTX], f32, tag="sm", name="sm")
                negm = mpool.tile([RSH, 1], f32, tag="negm", name="negm")
                esb = mpool.tile([RSH, N_CTX], f32, tag="esb", name="esb")
                den = mpool.tile([RSH, 1], f32, tag="den", name="den")
                if softmax_fused:
                    # nsm = negmask - s (masked cells ~ +1e30), negm =
                    # min(nsm) = -max over allowed, exp(-nsm + negm)
                    nc.vector.tensor_tensor_reduce(
                        out=nsm[:], in0=mask_sb[:], in1=sfull[:], scale=1.0,
                        scalar=float(-NEG_INF), op0=Alu.subtract, op1=Alu.min,
                        accum_out=negm[:])
                    nc.scalar.activation(
                        esb[:], nsm[:], Act.Exp, bias=negm[:], scale=-1.0,
                        accum_out=den[:])
                else:
                    # sm = s - negmask (masked cells ~ -1e30)
                    nc.vector.tensor_tensor(
                        out=nsm[:], in0=sfull[:], in1=mask_sb[:],
                        op=Alu.subtract)
                    nc.vector.reduce_max(
                        negm[:], nsm[:], axis=mybir.AxisListType.X,
                        negate=True)
                    nc.scalar.activation(
                        esb[:], nsm[:], Act.Exp, bias=negm[:], scale=1.0,
                        accum_out=den[:])
                rden = mpool.tile([RSH, 1], f32, tag="rden", name="rden")
                nc.vector.reciprocal(rden[:], den[:])

                # ---- A^T via PE transpose: [64, 512] -> 4x [128, 64] ------
                # (unnormalized exp weights; f32 transpose, and the
                # PSUM->SBUF copy casts to fp16 for the fp16 matmuls)
                at_sb = []
                for kt in range(4):
                    tp = ppB.tile([128, 512], f32, tag="tail", name="tp")
                    nc.tensor.transpose(
                        tp[:, 0:RSH],
                        esb[:, kt * 128:(kt + 1) * 128],
                        ident_sb[0:RSH, 0:RSH],
                    )
                    at = mpool.tile([128, RSH], f16, tag=f"at{kt}",
                                    name=f"at{kt}")
                    nc.scalar.copy(at[:], tp[:, 0:RSH])
                    at_sb.append(at)

                # ---- O^T = X^T @ A^T : [512(e), 64(i)] --------------------
                ot_sb = []
                for et in range(4):
                    op = ppB.tile([128, 512], f32, tag="tail", name="op")
                    for kt in range(4):
                        nc.tensor.matmul(
                            op[:, 0:RSH],
                            lhsT=xpwo_sb[:, kt, et * 128:(et + 1) * 128],
                            rhs=at_sb[kt][:],
                            start=(kt == 0),
                            stop=(kt == 3),
                        )
                    ot = mpool.tile([128, RSH], f16, tag=f"ot{et}",
                                    name=f"ot{et}")
                    nc.scalar.copy(ot[:], op[:, 0:RSH])
                    ot_sb.append(ot)

                # ---- Y = O @ W_out^T : [64(i), 512(f)] --------------------
                ypz = ppB.tile([128, 512], f32, tag="tail", name="ypz")
                for et in range(4):
                    nc.tensor.matmul(
                        ypz[0:RSH, :],
                        lhsT=ot_sb[et][:],
                        rhs=xpwo_sb[:, 4 + et, :],
                        start=(et == 0),
                        stop=(et == 3),
                    )
                # final copy normalizes the softmax: per-partition 1/den
                y_sb = mpool.tile([RSH, D], f32, tag="y_sb", name="y_sb")
                nc.scalar.mul(y_sb[:], ypz[0:RSH, :], rden[:])
                nc.sync.dma_start(out_t[:], y_sb[:])
                if debug_scores:
                    nc.sync.dma_start(dbg_t[:], scores_sb[:])

            if timing_loop:
                with tc.For_i(0, timing_loop, 1):
                    body()
            else:
                body()

    nc.compile()
    return nc


def _pack_upper(Wm):
    """[KSH, 512, 512] fp32 -> [KSH, 128, PACKW] fp16 upper-tri pack.

    U' = triu(W + W^T, 1) + diag(W); block dt holds U'[128dt+p, 128dt:512].
    """
    U = np.triu(Wm + Wm.transpose(0, 2, 1), 1)
    idx = np.arange(D)
    U[:, idx, idx] = Wm[:, idx, idx]
    pack = np.empty((KSH, 128, PACKW), np.float16)
    for dt in range(4):
        lo = 128 * dt
        pack[:, :, OFFS[dt]:OFFS[dt] + SPANS[dt]] = U[:, lo:lo + 128, lo:D]
    return pack


def _make_in_maps(x, W_bi, W_out, stage_a="causal"):
    x = np.ascontiguousarray(np.asarray(x, dtype=np.float32))
    W_bi = np.asarray(W_bi, dtype=np.float32)
    W_out = np.asarray(W_out, dtype=np.float32)
    # x row-major packed [p, nt, d] = x[128*nt + p, d]
    xpk = np.ascontiguousarray(x.reshape(4, 128, D).transpose(1, 0, 2))
    x16k = xpk.astype(np.float16)
    # xtp[p, dt, n] = x[n, 128*dt + p]
    xtp16 = np.ascontiguousarray(
        x.T.reshape(4, 128, N_CTX).transpose(1, 0, 2)).astype(np.float16)
    ident = np.eye(128, dtype=np.float32)
    # interleaved k-sharding: core m owns global columns k = 8*kk + m.
    # After the AllToAll gather, score column position p = r*64 + kk
    # holds global k = 8*kk + r, so X rows and the causal mask are
    # permuted to match.
    perm = np.array([8 * (p % KSH) + p // KSH for p in range(N_CTX)])
    xperm = x[perm]
    woutt = W_out.T
    # xpwo[p, 0:4, :] = xperm blocks, xpwo[p, 4+et, :] = W_out^T blocks
    xpwo = np.empty((128, 8, D), np.float16)
    xpwo[:, 0:4, :] = xperm.reshape(4, 128, D).transpose(1, 0, 2)
    xpwo[:, 4:8, :] = woutt.reshape(4, 128, D).transpose(1, 0, 2)
    kcol = perm[None, :]                       # global k at position p
    in_maps = []
    for m in range(NCORES):
        pack = _pack_upper(np.ascontiguousarray(W_bi[m::NCORES]))
        # pair layout: [j] = concat(pack[j], pack[63-j]) along the free dim
        pairs = np.concatenate([pack[:KSH // 2], pack[:KSH // 2 - 1:-1]],
                               axis=2)
        rows = np.arange(m * RSH, (m + 1) * RSH)[:, None]
        # negated mask: 0 where allowed, +1e30 where masked
        mask = np.where(kcol <= rows, 0.0, -NEG_INF).astype(np.float32)
        in_maps.append({
            "x": xpk,
            "x16": x16k,
            "xpwo": xpwo,
            "xtp": xtp16,
            "wbi": np.ascontiguousarray(pairs),
            "mask": np.ascontiguousarray(mask),
            "ident": ident,
        })
    return in_maps


def kernel(x, W_bi, W_out):
    global _nc_cache
    import time as _time
    from concourse.bass_utils import run_bass_kernel_spmd

    if _nc_cache is None:
        _nc_cache = _build(stage_a=STAGE_A)
    nc = _nc_cache
    in_maps = _make_in_maps(x, W_bi, W_out, stage_a=STAGE_A)
    last_exc = None
    for attempt in range(3):
        try:
            res = run_bass_kernel_spmd(nc, in_maps, core_ids=list(range(NCORES)),
                                       trace=False)
            break
        except Exception as e:  # transient NRT/axon wedges recover on retry
            last_exc = e
            _time.sleep(5.0 * (attempt + 1))
    else:
        raise last_exc
    out = np.concatenate([res.results[m]["out"] for m in range(NCORES)], axis=0)
    return np.ascontiguousarray(out, dtype=np.float32)


# revision 57
# speedup vs baseline: 1.7980x; 1.0085x over previous
"""Bilinear causal attention (nn_Attention_34772055228779) on 8 trn2 cores.

reference:
  scores[i,k] = x[i] @ W_bi[k] @ x[i]          [512, 512]
  attn = softmax(scores + causal_mask, axis=1)
  out  = (attn @ x) @ W_out.T                  [512, 512]

Device strategy (tensor-parallel over score columns, per sharding hint):
  core m holds the k-interleaved shard W_bi[m::8] (64 local columns).

  Only the symmetric part of W_bi[k] contributes to x^T W x, so the host
  packs U'_k = triu(W_k + W_k^T, 1) + diag(W_k)  (exact identity:
  x^T U' x = x^T W x).  U' is upper-triangular, so the d-row-block dt only
  has nonzeros in columns e >= 128*dt: the four matmul rhs spans are
  512/384/256/128 instead of 4x512 (37.5%% less PE work), and the packed
  fp16 stream is 320 KiB/k = 20 MiB/core instead of 64 MiB fp32.

  stage A: for each local k: Y_k = X16 @ U'16_k (fp16 matmuls, fp32 PSUM,
           lhsT = X^T fp16 resident), scores[:, k] = rowsum(Y_k * X32)
           via scalar_tensor_tensor, alternating DVE / Pool(gpsimd) so
           neither vector engine is the bottleneck.
  AllToAll over the [8 x 64-row, 64-col] score shard blocks: core m ends up
           with rows [64m, 64m+64) of the FULL score matrix.
  tail:    masked softmax rows (ACT exp with fused accum), A^T via fp16 PE
           transpose, O^T = X^T A^T, Y = O @ W_out^T (fp16), DMA 64 rows.
  host:    concatenates the 8 row blocks.
"""
import numpy as np

N_CTX = 512
D = 512
NCORES = 8
KSH = N_CTX // NCORES      # 64 score columns per core
RSH = N_CTX // NCORES      # 64 output rows per core
NEG_INF = -1e30
STAGE_A = "causal"   # "causal" skips fully-masked row-tiles (k-interleaved)

# upper-triangular pack: per dt row-block, columns [128*dt, 512)
SPANS = [512, 384, 256, 128]
OFFS = [0, 512, 896, 1152]          # column offset of block dt in the pack
PACKW = 1280                         # total packed width per partition

_nc_cache = None


def _build(timing_loop=0, use_collective=True, num_devices=NCORES,
           stage_a="causal", wbufs=4, stt_split=True, softmax_fused=False,
           gather_3d=True, debug_scores=False):
    # NOTE: softmax_fused=True (tensor_tensor_reduce min) compiles but
    # crashes the exec unit on real TRN2 hardware -- keep it off.
    """Build the Bass module.

    timing_loop=R>0 wraps the whole per-core body in a hardware For_i loop
    (R iterations) for slope timing; collectives can't sit in control flow,
    so timing variants pass use_collective=False (the gather DMA then reads
    the pre-collective buffer -- wrong data, identical shapes/costs).
    """
    import concourse.mybir as mybir
    import concourse.tile as tile
    from concourse import bacc

    f32 = mybir.dt.float32
    f16 = mybir.dt.float16
    Alu = mybir.AluOpType
    Act = mybir.ActivationFunctionType

    nc = bacc.Bacc(
        "TRN2", target_bir_lowering=False, debug=False,
        enable_asserts=False, num_devices=num_devices,
    )

    # x row-major packed [p, nt, d] in f32 and f16: one DMA each
    x_t = nc.dram_tensor("x", [128, 4, D], f32, kind="ExternalInput").ap()
    x16_t = nc.dram_tensor("x16", [128, 4, D], f16, kind="ExternalInput").ap()
    # tail constants packed [p, 8, e] f16: [:,0:4] = column-permuted X rows
    # (k-interleaved layout) for attn @ X, [:,4:8] = W_out^T blocks
    xpwo_t = nc.dram_tensor("xpwo", [128, 8, D], f16,
                            kind="ExternalInput").ap()
    # X^T packed [p, dt, n]: one DMA loads all four lhsT d-blocks
    xtp_t = nc.dram_tensor("xtp", [128, 4, N_CTX], f16,
                           kind="ExternalInput").ap()
    # W pairs: [j] holds packed U' for columns kk=j and kk=63-j
    wbi_t = nc.dram_tensor("wbi", [KSH // 2, 128, 2 * PACKW], f16,
                           kind="ExternalInput").ap()
    # negated additive mask: 0 where allowed, +1e30 where causally masked
    mask_t = nc.dram_tensor("mask", [RSH, N_CTX], f32, kind="ExternalInput").ap()
    out_t = nc.dram_tensor("out", [RSH, D], f32, kind="ExternalOutput").ap()
    dbg_t = (nc.dram_tensor("dbg", [128, 4 * KSH], f32,
                            kind="ExternalOutput").ap()
             if debug_scores else None)

    with tile.TileContext(nc) as tc:
        with (
            tc.tile_pool(name="const", bufs=1) as cpool,
            tc.tile_pool(name="tailc", bufs=2) as tcpool,
            tc.tile_pool(name="wstream", bufs=wbufs) as wpool,
            tc.tile_pool(name="scratch", bufs=3) as spool,
            tc.tile_pool(name="scratch2", bufs=3) as spool2,
            tc.tile_pool(name="small", bufs=1) as mpool,
            tc.tile_pool(name="psA", bufs=6, space="PSUM") as ppA,
            tc.tile_pool(name="psB", bufs=2, space="PSUM") as ppB,
            tc.tile_pool(name="dram", bufs=1, space="DRAM") as dpool,
        ):
            # ---- resident loads (outside any timing loop) -----------------
            # xt first (single packed DMA): the first matmul only needs
            # xt + wk0, so the x/x16 loads (needed ~2.6us later by the
            # first stt) are issued after the first wk DMA to cut the
            # startup serial chain.
            xtp_sb = cpool.tile([128, 4, N_CTX], f16, tag="xtp", name="xtp")
            nc.sync.dma_start(xtp_sb[:], xtp_t[:])
            xpk_sb = cpool.tile([128, 4, N_CTX], f32, tag="xpk", name="xpk")
            x16k_sb = cpool.tile([128, 4, N_CTX], f16, tag="x16k",
                                 name="x16k")

            def load_x_resident():
                nc.sync.dma_start(xpk_sb[:], x_t[:])
                nc.sync.dma_start(x16k_sb[:], x16_t[:])
            # single score accumulator tile, column nt*KSH + kk
            scores_sb = cpool.tile([128, 4 * KSH], f32, tag="sc", name="sc")
            # skipped (nt, kk) cells are never written; zero them so no
            # NaN bit-patterns survive into exp() past the additive mask
            nc.gpsimd.memset(scores_sb[:], 0.0)
            agin = dpool.tile([N_CTX, KSH], f32, tag="agin")
            agout = dpool.tile([N_CTX, KSH], f32, tag="agout")
            agin_v = agin[:].rearrange("(t p) k -> p t k", p=128)
            scores_v = scores_sb[:].rearrange("p (t k) -> p t k", t=4)

            def load_wk_pair(j):
                # one DMA covers both columns of the pair (j, 63-j)
                wk = wpool.tile([128, 2 * PACKW], f16, tag="wk", name="wk")
                nc.sync.dma_start(wk[:], wbi_t[j])
                return wk

            # stt engine split: only DVE can reduce straight from PSUM
            # (Pool has no PSUM access and TensorScalarPtr is not a legal
            # Pool opcode).  A share of tiles is therefore routed
            #   ACT:  yp (PSUM f32) -> y16 (SBUF f16)
            #   Pool: prod16 = y16 * x16          (TensorTensor, SBUF)
            #   ACT:  Copy(prod16) with accum_out -> scores column
            # Costs: DVE stt ~658 ns; ACT ~2x660 ns and Pool ~840 ns per
            # routed tile.  6 of every 20 tiles (spread, not consecutive,
            # so DVE never sits idle for long) puts DVE ~76us, ACT ~64us
            # and Pool ~40us, all under the ~90us PE stage-A floor.
            POOL_SLOTS = {3, 6, 9, 13, 16, 19}
            stt_state = {"i": 0}

            def emit_stt(yp, nt, kk):
                if stt_split:
                    use_dve = (stt_state["i"] % 20) not in POOL_SLOTS
                    stt_state["i"] += 1
                else:
                    use_dve = True
                col = nt * KSH + kk
                if use_dve:
                    scr = spool.tile([128, D], f32, tag="stt_out", name="scr")
                    nc.vector.scalar_tensor_tensor(
                        out=scr[:], in0=yp[:], scalar=1.0,
                        in1=xpk_sb[:, nt, :],
                        op0=Alu.mult, op1=Alu.mult,
                        accum_out=scores_sb[:, col:col + 1],
                    )
                else:
                    y16 = spool2.tile([128, D], f16, tag="y16", name="y16")
                    nc.scalar.copy(y16[:], yp[:])
                    prod = spool2.tile([128, D], f16, tag="prod", name="prod")
                    nc.gpsimd.tensor_tensor(
                        out=prod[:], in0=y16[:], in1=x16k_sb[:, nt, :],
                        op=Alu.mult)
                    scr = spool2.tile([128, D], f16, tag="scr16", name="scr16")
                    nc.scalar.activation(
                        scr[:], prod[:], Act.Copy, bias=0.0, scale=1.0,
                        accum_out=scores_sb[:, col:col + 1])

            def stage_a_tri(load_tail_consts):
                # causal: with k-interleaved sharding (global k = 8*kk + m),
                # row-tiles nt < kk//16 are fully masked for column kk on
                # EVERY core, so the skip bound is SPMD-uniform.
                #
                # Column order pairs kk with 63-kk: every pair is exactly 5
                # kept row-tiles of PE work against 2 wk DMAs, so the DMA
                # stream never outpaces nor starves the PE (a plain
                # ascending order leaves PE idle behind DMA for the late,
                # 1-tile columns).
                for j in range(KSH // 2):
                    wk = load_wk_pair(j)
                    if j == 0:
                        # must precede the first stt in program order: the
                        # dependency tracker only orders reads after writes
                        # that were already emitted
                        load_x_resident()
                    if j == 3:
                        load_tail_consts()
                    for half, kk in enumerate((j, KSH - 1 - j)):
                        base = half * PACKW
                        nt_lo = (kk // 16) if stage_a == "causal" else 0
                        for nt in range(nt_lo, 4):
                            yp = ppA.tile([128, D], f32, tag="yp", name="yp")
                            for dt in range(4):
                                span = SPANS[dt]
                                nc.tensor.matmul(
                                    yp[:, D - span:D],
                                    lhsT=xtp_sb[:, dt,
                                                nt * 128:(nt + 1) * 128],
                                    rhs=wk[:, base + OFFS[dt]:
                                           base + OFFS[dt] + span],
                                    start=(dt == 0),
                                    stop=(dt == 3),
                                    skip_group_check=True,
                                )
                            emit_stt(yp, nt, kk)
                    if j == 15 and gather_3d:
                        # columns {0..15, 48..63} are final: start their
                        # DRAM gather under the remaining compute.  On the
                        # Pool SWDGE queue so the wait on those columns'
                        # stts never blocks the SP weight-stream queue.
                        nc.gpsimd.dma_start(
                            agin_v[:, :, 0:16], scores_v[:, :, 0:16])
                        nc.gpsimd.dma_start(
                            agin_v[:, :, 48:64], scores_v[:, :, 48:64])

            def body():
                # tail constants, double-buffered (bufs=2) so the timing
                # loop's next iteration can re-load them without a
                # write-after-read stall against this iteration's tail
                tail_c = {}

                def load_tail_consts():
                    tail_c["xpwo"] = tcpool.tile(
                        [128, 8, N_CTX], f16, tag="xpwo", name="xpwo")
                    tail_c["mask"] = tcpool.tile(
                        [RSH, N_CTX], f32, tag="mask", name="mask")
                    nc.sync.dma_start(tail_c["xpwo"][:], xpwo_t[:])
                    nc.sync.dma_start(tail_c["mask"][:], mask_t[:])

                # ---- stage A: local score columns -------------------------
                stage_a_tri(load_tail_consts)
                xpwo_sb = tail_c["xpwo"]
                mask_sb = tail_c["mask"]

                # ---- AllToAll: shard columns -> shard rows ----------------
                # (columns {0..15, 48..63} were already gathered mid-stage-A)
                # Gather/scatter DMAs ride the Pool SWDGE queue, same as the
                # collective, keeping the SP queue free for the next
                # iteration's weight stream.
                if gather_3d:
                    nc.gpsimd.dma_start(
                        agin_v[:, :, 16:48], scores_v[:, :, 16:48])
                else:
                    for nt in range(4):
                        nc.gpsimd.dma_start(
                            agin[nt * 128:(nt + 1) * 128, :],
                            scores_sb[:, nt * KSH:(nt + 1) * KSH])
                if use_collective:
                    nc.gpsimd.collective_compute(
                        "AllToAll",
                        mybir.AluOpType.bypass,
                        replica_groups=[list(range(NCORES))],
                        ins=[agin[:].opt()],
                        outs=[agout[:].opt()],
                    )
                    coll_out = agout
                else:
                    coll_out = agin
                # rows of the full score matrix for this core: [64, 512]
                sfull = mpool.tile([RSH, N_CTX], f32, tag="sfull", name="sfull")
                nc.gpsimd.dma_start(
                    sfull[:].rearrange("i (r k) -> i r k", r=NCORES),
                    coll_out[:].rearrange("(r i) k -> i r k", r=NCORES),
                )

                # ---- masked softmax over the 64 rows ----------------------
                # fused mask+max: nsm = negmask - scores (so masked cells are
                # ~+1e30 and min(nsm) = -max of the allowed scores), then
                # exp(-nsm + bias) on ACT.  The 1/denominator is folded into
                # the final output copy as a per-partition ACT scale, keeping
                # the reciprocal off the critical path.
                nsm = mpool.tile([RSH, N_CTX], f32, tag="sm", name="sm")
                negm = mpool.tile([RSH, 1], f32, tag="negm", name="negm")
                esb = mpool.tile([RSH, N_CTX], f16, tag="esb", name="esb")
                den = mpool.tile([RSH, 1], f32, tag="den", name="den")
                if softmax_fused:
                    # nsm = negmask - s (masked cells ~ +1e30), negm =
                    # min(nsm) = -max over allowed, exp(-nsm + negm)
                    nc.vector.tensor_tensor_reduce(
                        out=nsm[:], in0=mask_sb[:], in1=sfull[:], scale=1.0,
                        scalar=float(-NEG_INF), op0=Alu.subtract, op1=Alu.min,
                        accum_out=negm[:])
                    nc.scalar.activation(
                        esb[:], nsm[:], Act.Exp, bias=negm[:], scale=-1.0,
                        accum_out=den[:])
                else:
                    # sm = s - negmask (masked cells ~ -1e30)
                    nc.vector.tensor_tensor(
                        out=nsm[:], in0=sfull[:], in1=mask_sb[:],
                        op=Alu.subtract)
                    nc.vector.reduce_max(
                        negm[:], nsm[:], axis=mybir.AxisListType.X,
                        negate=True)
                    nc.scalar.activation(
                        esb[:], nsm[:], Act.Exp, bias=negm[:], scale=1.0,
                        accum_out=den[:])
                rden = mpool.tile([RSH, 1], f32, tag="rden", name="rden")
                nc.vector.reciprocal(rden[:], den[:])

                # ---- A^T via xbar DMA transpose: [64, 512] -> 4x [128, 64]
                # (unnormalized fp16 exp weights; dispatched on the ACT
                # HWDGE queue so same-engine ordering after the exp makes
                # the chain wait-free)
                at_sb = []
                for kt in range(4):
                    at = mpool.tile([128, RSH], f16, tag=f"at{kt}",
                                    name=f"at{kt}")
                    nc.scalar.dma_start_transpose(
                        at[:], esb[:, kt * 128:(kt + 1) * 128])
                    at_sb.append(at)

                # ---- O^T = X^T @ A^T : [512(e), 64(i)] --------------------
                ot_sb = []
                for et in range(4):
                    op = ppB.tile([128, 512], f32, tag="tail", name="op")
                    for kt in range(4):
                        nc.tensor.matmul(
                            op[:, 0:RSH],
                            lhsT=xpwo_sb[:, kt, et * 128:(et + 1) * 128],
                            rhs=at_sb[kt][:],
                            start=(kt == 0),
                            stop=(kt == 3),
                        )
                    ot = mpool.tile([128, RSH], f16, tag=f"ot{et}",
                                    name=f"ot{et}")
                    nc.scalar.copy(ot[:], op[:, 0:RSH])
                    ot_sb.append(ot)

                # ---- Y = O @ W_out^T : [64(i), 512(f)] --------------------
                ypz = ppB.tile([128, 512], f32, tag="tail", name="ypz")
                for et in range(4):
                    nc.tensor.matmul(
                        ypz[0:RSH, :],
                        lhsT=ot_sb[et][:],
                        rhs=xpwo_sb[:, 4 + et, :],
                        start=(et == 0),
                        stop=(et == 3),
                    )
                # final copy normalizes the softmax: per-partition 1/den
                y_sb = mpool.tile([RSH, D], f32, tag="y_sb", name="y_sb")
                nc.scalar.mul(y_sb[:], ypz[0:RSH, :], rden[:])
                nc.scalar.dma_start(out_t[:], y_sb[:])
                if debug_scores:
                    nc.sync.dma_start(dbg_t[:], scores_sb[:])

            if timing_loop:
                with tc.For_i(0, timing_loop, 1):
                    body()
            else:
                body()

    nc.compile()
    return nc


def _pack_upper(Wm):
    """[KSH, 512, 512] fp32 -> [KSH, 128, PACKW] fp16 upper-tri pack.

    U' = triu(W + W^T, 1) + diag(W); block dt holds U'[128dt+p, 128dt:512].
    """
    U = np.triu(Wm + Wm.transpose(0, 2, 1), 1)
    idx = np.arange(D)
    U[:, idx, idx] = Wm[:, idx, idx]
    pack = np.empty((KSH, 128, PACKW), np.float16)
    for dt in range(4):
        lo = 128 * dt
        pack[:, :, OFFS[dt]:OFFS[dt] + SPANS[dt]] = U[:, lo:lo + 128, lo:D]
    return pack


def _make_in_maps(x, W_bi, W_out, stage_a="causal"):
    x = np.ascontiguousarray(np.asarray(x, dtype=np.float32))
    W_bi = np.asarray(W_bi, dtype=np.float32)
    W_out = np.asarray(W_out, dtype=np.float32)
    # x row-major packed [p, nt, d] = x[128*nt + p, d]
    xpk = np.ascontiguousarray(x.reshape(4, 128, D).transpose(1, 0, 2))
    x16k = xpk.astype(np.float16)
    # xtp[p, dt, n] = x[n, 128*dt + p]
    xtp16 = np.ascontiguousarray(
        x.T.reshape(4, 128, N_CTX).transpose(1, 0, 2)).astype(np.float16)
    # interleaved k-sharding: core m owns global columns k = 8*kk + m.
    # After the AllToAll gather, score column position p = r*64 + kk
    # holds global k = 8*kk + r, so X rows and the causal mask are
    # permuted to match.
    perm = np.array([8 * (p % KSH) + p // KSH for p in range(N_CTX)])
    xperm = x[perm]
    woutt = W_out.T
    # xpwo[p, 0:4, :] = xperm blocks, xpwo[p, 4+et, :] = W_out^T blocks
    xpwo = np.empty((128, 8, D), np.float16)
    xpwo[:, 0:4, :] = xperm.reshape(4, 128, D).transpose(1, 0, 2)
    xpwo[:, 4:8, :] = woutt.reshape(4, 128, D).transpose(1, 0, 2)
    kcol = perm[None, :]                       # global k at position p
    in_maps = []
    for m in range(NCORES):
        pack = _pack_upper(np.ascontiguousarray(W_bi[m::NCORES]))
        # pair layout: [j] = concat(pack[j], pack[63-j]) along the free dim
        pairs = np.concatenate([pack[:KSH // 2], pack[:KSH // 2 - 1:-1]],
                               axis=2)
        rows = np.arange(m * RSH, (m + 1) * RSH)[:, None]
        # negated mask: 0 where allowed, +1e30 where masked
        mask = np.where(kcol <= rows, 0.0, -NEG_INF).astype(np.float32)
        in_maps.append({
            "x": xpk,
            "x16": x16k,
            "xpwo": xpwo,
            "xtp": xtp16,
            "wbi": np.ascontiguousarray(pairs),
            "mask": np.ascontiguousarray(mask),
        })
    return in_maps


def kernel(x, W_bi, W_out):
    global _nc_cache
    import time as _time
    from concourse.bass_utils import run_bass_kernel_spmd

    if _nc_cache is None:
        _nc_cache = _build(stage_a=STAGE_A)
    nc = _nc_cache
    in_maps = _make_in_maps(x, W_bi, W_out, stage_a=STAGE_A)
    last_exc = None
    for attempt in range(3):
        try:
            res = run_bass_kernel_spmd(nc, in_maps, core_ids=list(range(NCORES)),
                                       trace=False)
            break
        except Exception as e:  # transient NRT/axon wedges recover on retry
            last_exc = e
            _time.sleep(5.0 * (attempt + 1))
    else:
        raise last_exc
    out = np.concatenate([res.results[m]["out"] for m in range(NCORES)], axis=0)
    return np.ascontiguousarray(out, dtype=np.float32)
